# revision 19
# baseline (speedup 1.0000x reference)
"""Trainium2 Bass kernel for nn_Decoder2 (dense transformer decoder block).

Sharding (8 cores):
  - both attentions: head-sharded, 2 heads (=128 feature dims) per core
  - FFN: hidden dim column/row sharded, 512 hidden units per core; the 8
    partial outputs are summed on the host
  - wemb/pemb replicated; all activations kept transposed [feat, seq]

The kernel is a software pipeline over 4 sequence chunks of 512: each
chunk's self-attention output is AllGathered independently, so the
collectives and the cross-attention/FFN for chunk c overlap the
self-attention of chunk c+1. Same for the cross->FFN boundary.

Matmuls run as float32r (TF32-like): full PE rate at free-dim>=256, ~1e-4
relative error. Tiles feeding fp32r matmuls are allocated float32r so the
producing engines round on write (BIR verifier requirement); DMA-loaded
tiles alias the fp32 DRAM bytes via bitcast (PE truncates on read).
Softmax is computed without max-subtraction (scores are O(+-6)); the
softmax denominator comes from a ones-column folded into the AV matmul
(lhsT = [v_head | ones], m=65). Scores for the two heads are issued
adjacently as K=64 row-tiles (tile_position) so they run concurrently.
"""

import numpy as np

import concourse.bass as bass
import concourse.bacc as bacc
import concourse.mybir as mybir
import concourse.tile as tile
from concourse.bass_utils import run_bass_kernel_spmd
from concourse.masks import make_identity

F32 = mybir.dt.float32
F32R = mybir.dt.float32r
AF = mybir.ActivationFunctionType

N_CORES = 8
S_W, S_P = 2048, 1024
D_MODEL, NEW_DIM, H, D_FF = 1024, 1024, 16, 4096
HD = 128          # head-feature dims per core (2 heads x 64)
FF_SH = D_FF // N_CORES   # 512 hidden units per core
NC = 512          # free-dim chunk for matmuls
DCH = D_MODEL // 128      # 8 contraction chunks of 128
NSQ = S_W // NC           # 4 sq chunks
NSKB = S_W // 128         # 16 self key blocks
NSPB = S_P // 128         # 8 cross key blocks
NFB = FF_SH // 128        # 4 ffn hidden blocks per core


def _rr(ap):
    """View an fp32 DRAM access as float32r (raw bytes, PE truncates on read)."""
    return ap.bitcast(F32R)


def decoder_kernel(tc):
    nc = tc.nc

    # all inputs host-prepacked to [128, ...] partition-major contiguous
    wembT = nc.dram_tensor("wembT", [128, NSQ * DCH * NC], F32,
                           kind="ExternalInput").ap()
    pembT = nc.dram_tensor("pembT", [128, 2 * DCH * NC], F32,
                           kind="ExternalInput").ap()
    wqmT = nc.dram_tensor("wqmT", [128, DCH * HD], F32, kind="ExternalInput").ap()
    wkmT = nc.dram_tensor("wkmT", [128, DCH * HD], F32, kind="ExternalInput").ap()
    wvmT = nc.dram_tensor("wvmT", [128, DCH * HD], F32, kind="ExternalInput").ap()
    wqcT = nc.dram_tensor("wqcT", [128, DCH * HD], F32, kind="ExternalInput").ap()
    wkcT = nc.dram_tensor("wkcT", [128, DCH * HD], F32, kind="ExternalInput").ap()
    wvcT = nc.dram_tensor("wvcT", [128, DCH * HD], F32, kind="ExternalInput").ap()
    w1T = nc.dram_tensor("w1T", [128, DCH * FF_SH], F32, kind="ExternalInput").ap()
    w2T = nc.dram_tensor("w2T", [128, DCH * NFB * 128], F32,
                         kind="ExternalInput").ap()
    outT = nc.dram_tensor("outT", [D_MODEL, S_W], F32, kind="ExternalOutput").ap()

    rg = [list(range(N_CORES))]

    with (
        tc.tile_pool(name="const", bufs=1) as constp,
        tc.tile_pool(name="dram", bufs=1, space="DRAM") as dramp,
        tc.tile_pool(name="big", bufs=1) as bigp,
        tc.tile_pool(name="chunk", bufs=2) as chkp,
        tc.tile_pool(name="work", bufs=2) as workp,
        tc.tile_pool(name="ps_pp", bufs=2, space="PSUM") as ps_pp,
        tc.tile_pool(name="ps_s", bufs=2, space="PSUM") as ps_s,
        tc.tile_pool(name="ps_o", bufs=1, space="PSUM") as ps_o,
    ):
        # ---- constants ----
        ident = constp.tile([128, 128], F32, tag="ident")
        make_identity(nc, ident[:])
        ones_col = constp.tile([128, 1], F32, tag="ones_col")
        nc.vector.memset(ones_col[:], 1.0)
        # extended causal mask: mask_ext[x, yy] = 1 iff yy - x >= 384.
        # view k (k=0..3): mask_ext[:, 384-128k : 896-128k] gives
        # [x, y] = 1 iff y - x >= 128k.
        mask_ext = constp.tile([128, 896], F32, tag="mask_ext")
        nc.gpsimd.memset(mask_ext[:], 1.0)
        nc.gpsimd.affine_select(
            out=mask_ext[:], in_=mask_ext[:],
            compare_op=mybir.AluOpType.is_ge,
            fill=0.0,
            base=-384,
            pattern=[[1, 896]],
            channel_multiplier=-1,
        )

        def mask_view(k):
            return mask_ext[:, 384 - 128 * k:896 - 128 * k]

        # ---- weight loads (tags reused self->cross) ----
        def load_wT(dram_ap, tag, name):
            t = constp.tile([128, DCH * HD], F32R, tag=tag, name=name)
            nc.sync.dma_start(t[:], _rr(dram_ap))
            return t

        # warmup collective: absorbs the ~60us first-collective ncfw setup
        # cost while the projections run
        warm_in = dramp.tile([128, 4], F32, name="warm_in")
        warm_out = dramp.tile([N_CORES * 128, 4], F32, name="warm_out",
                              addr_space="Shared")
        warm_sb = constp.tile([128, 4], F32, tag="warm_sb")
        nc.vector.memset(warm_sb[:], 0.0)
        nc.sync.dma_start(warm_in[:], warm_sb[:])
        nc.gpsimd.collective_compute(
            "AllGather",
            mybir.AluOpType.bypass,
            replica_groups=rg,
            ins=[warm_in[:].opt()],
            outs=[warm_out[:].opt()],
        )

        wq_sb = load_wT(wqmT, "wq", "wqm")
        wk_sb = load_wT(wkmT, "wk", "wkm")
        wv_sb = load_wT(wvmT, "wv", "wvm")

        # ---- self qkv projections, chunked over seq ----
        qT = bigp.tile([128, S_W], F32R, tag="qT", name="qT")
        kT = bigp.tile([128, S_W], F32R, tag="kT", name="kT")
        v65 = bigp.tile([128, NSKB * 130], F32R, tag="v65", name="v65")

        def proj_chunk(out_ap, w_sb, x_chunks, dtype_note=None):
            ps = ps_pp.tile([128, NC], F32, tag="pp", name="ps_pj")
            for dc in range(DCH):
                nc.tensor.matmul(
                    ps[:],
                    w_sb[:, HD * dc:HD * (dc + 1)],
                    x_chunks[dc][:],
                    start=(dc == 0),
                    stop=(dc == DCH - 1),
                )
            nc.vector.tensor_copy(out_ap, ps[:])

        def transp_block(v65_sb, vt_c, lb, b):
            ps = ps_pp.tile([128, 128], F32, tag="pp", name="ps_tr")
            nc.tensor.transpose(ps[:], vt_c[:, 128 * lb:128 * (lb + 1)], ident[:])
            nc.vector.tensor_copy(v65_sb[:, 130 * b:130 * b + 64], ps[:, 0:64])
            nc.vector.tensor_copy(
                v65_sb[:, 130 * b + 65:130 * b + 129], ps[:, 64:128])
            nc.vector.tensor_copy(v65_sb[:, 130 * b + 64:130 * b + 65], ones_col[:])
            nc.vector.tensor_copy(
                v65_sb[:, 130 * b + 129:130 * b + 130], ones_col[:])


        # ---- attention chunk helper ----
        # Per j-step: both heads' scores go into one [128,1024] PSUM pair
        # (adjacent K=64 row-tiles, concurrent), ONE exp over both, then two
        # m=65 AV matmuls (ones-column -> softmax denominator in row 64).
        # `filler` emits one unit of independent PE work after each j-step to
        # keep the PE dense (HAM warm) through the ACT-bound exp chain.
        def attention_chunk(out_c, q_ap, k_sb, v65_sb, n_j, causal_c,
                            fillers=()):
            fill = iter(fillers)
            pso = [ps_o.tile([65, NC], F32, tag=f"o{h}", name=f"pso{h}")
                   for h in range(2)]
            for j in range(n_j):
                pss = ps_s.tile([128, 2 * NC], F32, tag="s", name="pss")
                for h in range(2):
                    nc.tensor.matmul(
                        pss[:, NC * h:NC * (h + 1)],
                        k_sb[64 * h:64 * (h + 1), 128 * j:128 * (j + 1)],
                        q_ap[64 * h:64 * (h + 1), :],
                        start=True, stop=True,
                        tile_position=(64 * h, 0),
                    )
                es = workp.tile([128, 2 * NC], F32R, tag="e", name="es")
                nc.scalar.activation(es[:], pss[:], AF.Exp, scale=0.125)
                if causal_c is not None and j >= 4 * causal_c:
                    for h in range(2):
                        nc.vector.tensor_mul(
                            es[:, NC * h:NC * (h + 1)],
                            es[:, NC * h:NC * (h + 1)],
                            mask_view(j - 4 * causal_c),
                        )
                for h in range(2):
                    nc.tensor.matmul(
                        pso[h][:],
                        v65_sb[:, 130 * j + 65 * h:130 * j + 65 * h + 65],
                        es[:, NC * h:NC * (h + 1)],
                        start=(j == 0),
                        stop=(j == n_j - 1),
                    )
                for th in (next(fill, None),):
                    if th is not None:
                        th()
            for th in fill:
                th()
            for h in range(2):
                lrow = workp.tile([1, NC], F32, tag="lrow", name="lrow")
                nc.vector.tensor_copy(lrow[:], pso[h][64:65, :])
                rec = workp.tile([1, NC], F32, tag="rec", name="rec")
                nc.vector.reciprocal_approx_fast(rec[:], lrow[:])
                rec64 = workp.tile([64, NC], F32, tag="rec64", name="rec64")
                nc.gpsimd.partition_broadcast(rec64[:], rec[:])
                nc.vector.tensor_mul(
                    out_c[64 * h:64 * (h + 1), :], pso[h][0:64, :], rec64[:])

        # ---- work-unit emitters (used as attention fillers) ----
        wqc_sb = load_wT(wqcT, "wq2", "wqc")
        wkc_sb = load_wT(wkcT, "wk2", "wkc")
        wvc_sb = load_wT(wvcT, "wv2", "wvc")
        kcT = bigp.tile([128, S_P], F32R, tag="kcT", name="kcT")
        vc65 = bigp.tile([128, NSPB * 130], F32R, tag="vc65", name="vc65")
        wd_c = []
        cd_c = {}
        qc_t = {}

        def xcat_load(dram_ap, name):
            """prepacked [128, 8*512] DRAM block -> SBUF tile, one DMA."""
            t = chkp.tile([128, DCH * NC], F32R, tag="xcat", name=name)
            nc.sync.dma_start(t[:], _rr(dram_ap))
            return [t[:, NC * dc:NC * (dc + 1)] for dc in range(DCH)]

        def dma_wemb(c):
            return xcat_load(
                wembT[:, DCH * NC * c:DCH * NC * (c + 1)], f"wemb_{c}")

        def proj_q(c, xc):
            proj_chunk(qT[:, NC * c:NC * (c + 1)], wq_sb, xc)

        def proj_k(c, xc):
            proj_chunk(kT[:, NC * c:NC * (c + 1)], wk_sb, xc)

        def proj_v(c, xc):
            vtc = chkp.tile([128, NC], F32, tag="vt", name=f"vT{c}", bufs=3)
            proj_chunk(vtc[:], wv_sb, xc)
            for lb in range(4):
                transp_block(v65, vtc, lb, 4 * c + lb)

        def proj_kc(sc, xc):
            proj_chunk(kcT[:, NC * sc:NC * (sc + 1)], wkc_sb, xc)

        def proj_vc(sc, xc):
            vtc = chkp.tile([128, NC], F32, tag="vt", name=f"vcT{sc}", bufs=3)
            proj_chunk(vtc[:], wvc_sb, xc)
            for lb in range(4):
                transp_block(vc65, vtc, lb, 4 * sc + lb)

        def qc_proj(c):
            t = chkp.tile([128, DCH * NC], F32R, tag="wdcat",
                          name=f"word_{c}", bufs=2)
            nc.gpsimd.dma_start(
                t[:].rearrange("p (u m) -> p u m", u=DCH),
                _rr(wd_c[c][:].rearrange("(u p) m -> p u m", p=128)),
            )
            xw = [t[:, NC * dc:NC * (dc + 1)] for dc in range(DCH)]
            qc = chkp.tile([128, NC], F32R, tag=f"qc{c % 2}", name=f"qcT{c}")
            proj_chunk(qc[:], wqc_sb, xw)
            qc_t[c] = qc

        def allgather(src_sb, name):
            bounce = dramp.tile([128, NC], F32, name=f"bnc_{name}")
            gath = dramp.tile([N_CORES * 128, NC], F32, name=f"gd_{name}",
                              addr_space="Shared")
            nc.sync.dma_start(bounce[:], src_sb[:])
            nc.gpsimd.collective_compute(
                "AllGather",
                mybir.AluOpType.bypass,
                replica_groups=rg,
                ins=[bounce[:].opt()],
                outs=[gath[:].opt()],
            )
            return gath

        # FFN weights (full resident)
        w1_sb = constp.tile([128, DCH * FF_SH], F32R, tag="w1", name="w1")
        nc.sync.dma_start(w1_sb[:], _rr(w1T))

        ffn_state = {}

        def ffn_load(c):
            t = chkp.tile([128, DCH * NC], F32R, tag="xcat", name=f"cr_{c}")
            nc.gpsimd.dma_start(
                t[:].rearrange("p (u m) -> p u m", u=DCH),
                _rr(cd_c[c][:].rearrange("(u p) m -> p u m", p=128)),
            )
            xc = [t[:, NC * dc:NC * (dc + 1)] for dc in range(DCH)]
            ffn_state[c] = (xc, [])

        def ffn1(c, fb):
            xc, hts = ffn_state[c]
            ps = ps_pp.tile([128, NC], F32, tag="pp", name="ps_f1")
            for dc in range(DCH):
                nc.tensor.matmul(
                    ps[:],
                    w1_sb[:, FF_SH * dc + 128 * fb:FF_SH * dc + 128 * (fb + 1)],
                    xc[dc][:],
                    start=(dc == 0),
                    stop=(dc == DCH - 1),
                )
            ht = chkp.tile([128, NC], F32R, tag=f"h{fb}", name=f"hT{fb}_{c}", bufs=1)
            nc.vector.tensor_relu(ht[:], ps[:])
            hts.append(ht)

        def ffn2(c, ob):
            hts = ffn_state[c][1]
            w2f = workp.tile([128, NFB * 128], F32R, tag="w2f", name="w2f")
            nc.sync.dma_start(
                w2f[:],
                _rr(w2T[:, NFB * 128 * ob:NFB * 128 * (ob + 1)]),
            )
            ps = ps_pp.tile([128, NC], F32, tag="pp", name="ps_f2")
            for fc in range(NFB):
                nc.tensor.matmul(
                    ps[:],
                    w2f[:, 128 * fc:128 * (fc + 1)],
                    hts[fc][:],
                    start=(fc == 0),
                    stop=(fc == NFB - 1),
                )
            o_sb = workp.tile([128, NC], F32, tag="o_sb", name="o_sb")
            nc.vector.tensor_copy(o_sb[:], ps[:])
            nc.gpsimd.dma_start(
                outT[128 * ob:128 * (ob + 1), NC * c:NC * (c + 1)], o_sb[:])

        def ffn_thunks(c):
            ts = [lambda c=c: ffn_load(c)]
            ts += [lambda c=c, fb=fb: ffn1(c, fb) for fb in range(NFB)]
            ts += [lambda c=c, ob=ob: ffn2(c, ob) for ob in range(DCH)]
            return ts

        def pemb_chunks(sc):
            return xcat_load(
                pembT[:, DCH * NC * sc:DCH * NC * (sc + 1)], f"pemb_{sc}")

        # ---- the pipeline ----
        # prologue: projections for self chunk 0
        xc0 = dma_wemb(0)
        proj_q(0, xc0)
        proj_k(0, xc0)
        proj_v(0, xc0)

        # filler schedules per self chunk (n_j = 4, 8, 12, 16)
        def self_fillers(c):
            ts = []
            if c + 1 < NSQ:
                xcn = dma_wemb(c + 1)
                ts += [lambda: proj_q(c + 1, xcn),
                       lambda: proj_k(c + 1, xcn),
                       lambda: proj_v(c + 1, xcn)]
            if c == 1:
                xp0 = pemb_chunks(0)
                ts += [lambda: proj_kc(0, xp0), lambda: proj_vc(0, xp0)]
                xp1 = pemb_chunks(1)
                ts += [lambda: proj_kc(1, xp1), lambda: proj_vc(1, xp1)]
            if c == 2:
                ts += [lambda: qc_proj(0)]
            if c == 3:
                ts += [lambda: qc_proj(1), lambda: qc_proj(2)]
            return ts

        for c in range(NSQ):
            self_c = chkp.tile([128, NC], F32, tag=f"oa{c % 2}", name=f"selfT{c}")
            attention_chunk(self_c[:], qT[:, NC * c:NC * (c + 1)], kT, v65,
                            4 * (c + 1), causal_c=c, fillers=self_fillers(c))
            wd_c.append(allgather(self_c, f"w{c}"))

        # cross chunks with qc/FFN fillers
        def cross_fillers(c):
            ts = []
            if c == 0:
                ts += [lambda: qc_proj(3)]
            if c >= 2:
                ts += ffn_thunks(c - 2)
            return ts

        for c in range(NSQ):
            cross_c = chkp.tile([128, NC], F32, tag=f"oa{c % 2}",
                                name=f"crossT{c}")
            attention_chunk(cross_c[:], qc_t[c][:], kcT, vc65, NSPB,
                            causal_c=None, fillers=cross_fillers(c))
            cd_c[c] = allgather(cross_c, f"c{c}")

        for th in ffn_thunks(NSQ - 2):
            th()
        for th in ffn_thunks(NSQ - 1):
            th()


_CACHED_NC = None


def _build():
    global _CACHED_NC
    if _CACHED_NC is None:
        nc = bacc.Bacc(
            "TRN2",
            target_bir_lowering=False,
            debug=False,
            num_devices=N_CORES,
        )
        with tile.TileContext(nc) as tc:
            decoder_kernel(tc)
        nc.compile()
        _CACHED_NC = nc
    return _CACHED_NC


def _pack_w(wT):
    """[1024, m] -> [128, 8*m]: d-chunk blocks side by side, partition-major."""
    m = wT.shape[1]
    return np.ascontiguousarray(
        wT.reshape(8, 128, m).transpose(1, 0, 2).reshape(128, 8 * m)
        .astype(np.float32))


def _pack_x(xT, nch):
    """[1024, nch*512] -> [128, nch * 8 * 512]: per seq-chunk c, the 8
    feature-blocks of that chunk's columns, contiguous."""
    return np.ascontiguousarray(
        xT.reshape(8, 128, nch, 512).transpose(1, 2, 0, 3)
        .reshape(128, nch * 8 * 512).astype(np.float32))


def make_in_maps(inputs):
    """Host-side prep: transposes + per-core weight slices + prepack."""
    f = np.ascontiguousarray
    wembT = _pack_x(np.asarray(inputs["wemb"], np.float32).T, NSQ)
    pembT = _pack_x(np.asarray(inputs["pemb"], np.float32).T, 2)
    in_maps = []
    for i in range(N_CORES):
        hsl = slice(HD * i, HD * (i + 1))
        fsl = slice(FF_SH * i, FF_SH * (i + 1))
        w2T = np.asarray(inputs["W2"], np.float32)[:, fsl].T  # [512, 1024]
        w2h = f(w2T.reshape(4, 128, 8, 128).transpose(1, 2, 0, 3)
                .reshape(128, 4096))
        in_maps.append({
            "wembT": wembT,
            "pembT": pembT,
            "wqmT": _pack_w(np.asarray(inputs["Wq_m"], np.float32)[hsl, :].T),
            "wkmT": _pack_w(np.asarray(inputs["Wk_m"], np.float32)[hsl, :].T),
            "wvmT": _pack_w(np.asarray(inputs["Wv_m"], np.float32)[hsl, :].T),
            "wqcT": _pack_w(np.asarray(inputs["Wq_c"], np.float32)[hsl, :].T),
            "wkcT": _pack_w(np.asarray(inputs["Wk_c"], np.float32)[hsl, :].T),
            "wvcT": _pack_w(np.asarray(inputs["Wv_c"], np.float32)[hsl, :].T),
            "w1T": _pack_w(np.asarray(inputs["W1"], np.float32)[fsl, :].T),
            "w2T": w2h,
        })
    return in_maps


def kernel(**inputs) -> np.ndarray:
    nc = _build()
    in_maps = make_in_maps(inputs)
    res = run_bass_kernel_spmd(nc, in_maps, core_ids=list(range(N_CORES)))
    acc = np.zeros((D_MODEL, S_W), dtype=np.float64)
    for i in range(N_CORES):
        acc += res.results[i]["outT"]
    return np.ascontiguousarray(acc.T.astype(np.float32))


# revision 20
# speedup vs baseline: 1.0433x; 1.0433x over previous
"""Trainium2 Bass kernel for nn_Decoder2 (dense transformer decoder block).

Sharding (8 cores):
  - both attentions: head-sharded, 2 heads (=128 feature dims) per core
  - FFN: hidden dim column/row sharded, 512 hidden units per core; the 8
    partial outputs are summed on the host
  - wemb/pemb replicated; all activations kept transposed [feat, seq]

The kernel is a software pipeline over 4 sequence chunks of 512: each
chunk's self-attention output is AllGathered independently, so the
collectives and the cross-attention/FFN for chunk c overlap the
self-attention of chunk c+1. Same for the cross->FFN boundary.

Matmuls run as float32r (TF32-like): full PE rate at free-dim>=256, ~1e-4
relative error. Tiles feeding fp32r matmuls are allocated float32r so the
producing engines round on write (BIR verifier requirement); DMA-loaded
tiles alias the fp32 DRAM bytes via bitcast (PE truncates on read).
Softmax is computed without max-subtraction (scores are O(+-6)); the
softmax denominator comes from a ones-column folded into the AV matmul
(lhsT = [v_head | ones], m=65). Scores for the two heads are issued
adjacently as K=64 row-tiles (tile_position) so they run concurrently.
"""

import numpy as np

import concourse.bass as bass
import concourse.bacc as bacc
import concourse.mybir as mybir
import concourse.tile as tile
from concourse.bass_utils import run_bass_kernel_spmd
from concourse.masks import make_identity

F32 = mybir.dt.float32
F32R = mybir.dt.float32r
AF = mybir.ActivationFunctionType

N_CORES = 8
S_W, S_P = 2048, 1024
D_MODEL, NEW_DIM, H, D_FF = 1024, 1024, 16, 4096
HD = 128          # head-feature dims per core (2 heads x 64)
FF_SH = D_FF // N_CORES   # 512 hidden units per core
NC = 512          # free-dim chunk for matmuls
DCH = D_MODEL // 128      # 8 contraction chunks of 128
NSQ = S_W // NC           # 4 sq chunks
NSKB = S_W // 128         # 16 self key blocks
NSPB = S_P // 128         # 8 cross key blocks
NFB = FF_SH // 128        # 4 ffn hidden blocks per core


def _rr(ap):
    """View an fp32 DRAM access as float32r (raw bytes, PE truncates on read)."""
    return ap.bitcast(F32R)


def decoder_kernel(tc):
    nc = tc.nc

    # all inputs host-prepacked to [128, ...] partition-major contiguous
    wembT = nc.dram_tensor("wembT", [128, NSQ * DCH * NC], F32,
                           kind="ExternalInput").ap()
    pembT = nc.dram_tensor("pembT", [128, 2 * DCH * NC], F32,
                           kind="ExternalInput").ap()
    wqmT = nc.dram_tensor("wqmT", [128, DCH * HD], F32, kind="ExternalInput").ap()
    wkmT = nc.dram_tensor("wkmT", [128, DCH * HD], F32, kind="ExternalInput").ap()
    wvmT = nc.dram_tensor("wvmT", [128, DCH * HD], F32, kind="ExternalInput").ap()
    wqcT = nc.dram_tensor("wqcT", [128, DCH * HD], F32, kind="ExternalInput").ap()
    wkcT = nc.dram_tensor("wkcT", [128, DCH * HD], F32, kind="ExternalInput").ap()
    wvcT = nc.dram_tensor("wvcT", [128, DCH * HD], F32, kind="ExternalInput").ap()
    w1T = nc.dram_tensor("w1T", [128, DCH * FF_SH], F32, kind="ExternalInput").ap()
    w2T = nc.dram_tensor("w2T", [128, DCH * NFB * 128], F32,
                         kind="ExternalInput").ap()
    outT = nc.dram_tensor("outT", [D_MODEL, S_W], F32, kind="ExternalOutput").ap()

    rg = [list(range(N_CORES))]

    with (
        tc.tile_pool(name="const", bufs=1) as constp,
        tc.tile_pool(name="dram", bufs=1, space="DRAM") as dramp,
        tc.tile_pool(name="big", bufs=1) as bigp,
        tc.tile_pool(name="chunk", bufs=2) as chkp,
        tc.tile_pool(name="work", bufs=2) as workp,
        tc.tile_pool(name="ps_pp", bufs=2, space="PSUM") as ps_pp,
        tc.tile_pool(name="ps_s", bufs=2, space="PSUM") as ps_s,
        tc.tile_pool(name="ps_o", bufs=1, space="PSUM") as ps_o,
    ):
        # ---- constants ----
        ident = constp.tile([128, 128], F32, tag="ident")
        make_identity(nc, ident[:])
        ones_col = constp.tile([128, 1], F32, tag="ones_col")
        nc.vector.memset(ones_col[:], 1.0)
        # extended causal mask: mask_ext[x, yy] = 1 iff yy - x >= 384.
        # view k (k=0..3): mask_ext[:, 384-128k : 896-128k] gives
        # [x, y] = 1 iff y - x >= 128k.
        mask_ext = constp.tile([128, 896], F32, tag="mask_ext")
        nc.gpsimd.memset(mask_ext[:], 1.0)
        nc.gpsimd.affine_select(
            out=mask_ext[:], in_=mask_ext[:],
            compare_op=mybir.AluOpType.is_ge,
            fill=0.0,
            base=-384,
            pattern=[[1, 896]],
            channel_multiplier=-1,
        )

        def mask_view(k):
            return mask_ext[:, 384 - 128 * k:896 - 128 * k]

        # ---- weight loads (tags reused self->cross) ----
        def load_wT(dram_ap, tag, name):
            t = constp.tile([128, DCH * HD], F32R, tag=tag, name=name)
            nc.sync.dma_start(t[:], _rr(dram_ap))
            return t

        # warmup collective: absorbs the ~60us first-collective ncfw setup
        # cost while the projections run
        warm_in = dramp.tile([128, 4], F32, name="warm_in")
        warm_out = dramp.tile([N_CORES * 128, 4], F32, name="warm_out",
                              addr_space="Shared")
        warm_sb = constp.tile([128, 4], F32, tag="warm_sb")
        nc.vector.memset(warm_sb[:], 0.0)
        nc.sync.dma_start(warm_in[:], warm_sb[:])
        nc.gpsimd.collective_compute(
            "AllGather",
            mybir.AluOpType.bypass,
            replica_groups=rg,
            ins=[warm_in[:].opt()],
            outs=[warm_out[:].opt()],
        )

        wq_sb = load_wT(wqmT, "wq", "wqm")
        wk_sb = load_wT(wkmT, "wk", "wkm")
        wv_sb = load_wT(wvmT, "wv", "wvm")

        # ---- self qkv projections, chunked over seq ----
        qT = bigp.tile([128, S_W], F32R, tag="qT", name="qT")
        kT = bigp.tile([128, S_W], F32R, tag="kT", name="kT")
        v65 = bigp.tile([128, NSKB * 130], F32R, tag="v65", name="v65")

        def proj_chunk(out_ap, w_sb, x_chunks, dtype_note=None):
            ps = ps_pp.tile([128, NC], F32, tag="pp", name="ps_pj")
            for dc in range(DCH):
                nc.tensor.matmul(
                    ps[:],
                    w_sb[:, HD * dc:HD * (dc + 1)],
                    x_chunks[dc][:],
                    start=(dc == 0),
                    stop=(dc == DCH - 1),
                )
            nc.vector.tensor_copy(out_ap, ps[:])

        def transp_block(v65_sb, vt_c, lb, b):
            ps = ps_pp.tile([128, 128], F32, tag="pp", name="ps_tr")
            nc.tensor.transpose(ps[:], vt_c[:, 128 * lb:128 * (lb + 1)], ident[:])
            nc.vector.tensor_copy(v65_sb[:, 130 * b:130 * b + 64], ps[:, 0:64])
            nc.vector.tensor_copy(
                v65_sb[:, 130 * b + 65:130 * b + 129], ps[:, 64:128])
            nc.vector.tensor_copy(v65_sb[:, 130 * b + 64:130 * b + 65], ones_col[:])
            nc.vector.tensor_copy(
                v65_sb[:, 130 * b + 129:130 * b + 130], ones_col[:])


        # ---- attention chunk helper ----
        # Per j-step: both heads' scores go into one [128,1024] PSUM pair
        # (adjacent K=64 row-tiles, concurrent), ONE exp over both, then two
        # m=65 AV matmuls (ones-column -> softmax denominator in row 64).
        # `filler` emits one unit of independent PE work after each j-step to
        # keep the PE dense (HAM warm) through the ACT-bound exp chain.
        def attention_chunk(out_c, q_ap, k_sb, v65_sb, n_j, causal_c,
                            fillers=()):
            fill = iter(fillers)
            pso = [ps_o.tile([65, NC], F32, tag=f"o{h}", name=f"pso{h}")
                   for h in range(2)]
            for j in range(n_j):
                pss = ps_s.tile([128, 2 * NC], F32, tag="s", name="pss")
                for h in range(2):
                    nc.tensor.matmul(
                        pss[:, NC * h:NC * (h + 1)],
                        k_sb[64 * h:64 * (h + 1), 128 * j:128 * (j + 1)],
                        q_ap[64 * h:64 * (h + 1), :],
                        start=True, stop=True,
                        tile_position=(64 * h, 0),
                    )
                es = workp.tile([128, 2 * NC], F32R, tag="e", name="es")
                nc.scalar.activation(es[:], pss[:], AF.Exp, scale=0.125)
                if causal_c is not None and j >= 4 * causal_c:
                    for h in range(2):
                        nc.vector.tensor_mul(
                            es[:, NC * h:NC * (h + 1)],
                            es[:, NC * h:NC * (h + 1)],
                            mask_view(j - 4 * causal_c),
                        )
                for h in range(2):
                    nc.tensor.matmul(
                        pso[h][:],
                        v65_sb[:, 130 * j + 65 * h:130 * j + 65 * h + 65],
                        es[:, NC * h:NC * (h + 1)],
                        start=(j == 0),
                        stop=(j == n_j - 1),
                    )
                for th in (next(fill, None),):
                    if th is not None:
                        th()
            for th in fill:
                th()
            for h in range(2):
                lrow = workp.tile([1, NC], F32, tag="lrow", name="lrow")
                nc.vector.tensor_copy(lrow[:], pso[h][64:65, :])
                rec = workp.tile([1, NC], F32, tag="rec", name="rec")
                nc.vector.reciprocal_approx_fast(rec[:], lrow[:])
                rec64 = workp.tile([64, NC], F32, tag="rec64", name="rec64")
                nc.gpsimd.partition_broadcast(rec64[:], rec[:])
                nc.vector.tensor_mul(
                    out_c[64 * h:64 * (h + 1), :], pso[h][0:64, :], rec64[:])

        # ---- work-unit emitters (used as attention fillers) ----
        wqc_sb = load_wT(wqcT, "wq2", "wqc")
        wkc_sb = load_wT(wkcT, "wk2", "wkc")
        wvc_sb = load_wT(wvcT, "wv2", "wvc")
        kcT = bigp.tile([128, S_P], F32R, tag="kcT", name="kcT")
        vc65 = bigp.tile([128, NSPB * 130], F32R, tag="vc65", name="vc65")
        wd_c = []
        cd_c = {}
        qc_t = {}

        def xcat_load(dram_ap, name):
            """prepacked [128, 8*512] DRAM block -> SBUF tile, one DMA."""
            t = chkp.tile([128, DCH * NC], F32R, tag="xcat", name=name)
            nc.sync.dma_start(t[:], _rr(dram_ap))
            return [t[:, NC * dc:NC * (dc + 1)] for dc in range(DCH)]

        def dma_wemb(c):
            return xcat_load(
                wembT[:, DCH * NC * c:DCH * NC * (c + 1)], f"wemb_{c}")

        def proj_q(c, xc):
            proj_chunk(qT[:, NC * c:NC * (c + 1)], wq_sb, xc)

        def proj_k(c, xc):
            proj_chunk(kT[:, NC * c:NC * (c + 1)], wk_sb, xc)

        def proj_v(c, xc):
            vtc = chkp.tile([128, NC], F32, tag="vt", name=f"vT{c}", bufs=3)
            proj_chunk(vtc[:], wv_sb, xc)
            for lb in range(4):
                transp_block(v65, vtc, lb, 4 * c + lb)

        def proj_kc(sc, xc):
            proj_chunk(kcT[:, NC * sc:NC * (sc + 1)], wkc_sb, xc)

        def proj_vc(sc, xc):
            vtc = chkp.tile([128, NC], F32, tag="vt", name=f"vcT{sc}", bufs=3)
            proj_chunk(vtc[:], wvc_sb, xc)
            for lb in range(4):
                transp_block(vc65, vtc, lb, 4 * sc + lb)

        def qc_proj(c):
            t = chkp.tile([128, DCH * NC], F32R, tag="wdcat",
                          name=f"word_{c}", bufs=2)
            nc.sync.dma_start(
                t[:].rearrange("p (u m) -> p u m", u=DCH),
                _rr(wd_c[c][:].rearrange("(u p) m -> p u m", p=128)),
            )
            xw = [t[:, NC * dc:NC * (dc + 1)] for dc in range(DCH)]
            qc = chkp.tile([128, NC], F32R, tag=f"qc{c % 2}", name=f"qcT{c}")
            proj_chunk(qc[:], wqc_sb, xw)
            qc_t[c] = qc

        def allgather(src_sb, name):
            bounce = dramp.tile([128, NC], F32, name=f"bnc_{name}")
            gath = dramp.tile([N_CORES * 128, NC], F32, name=f"gd_{name}",
                              addr_space="Shared")
            nc.sync.dma_start(bounce[:], src_sb[:])
            nc.gpsimd.collective_compute(
                "AllGather",
                mybir.AluOpType.bypass,
                replica_groups=rg,
                ins=[bounce[:].opt()],
                outs=[gath[:].opt()],
            )
            return gath

        # FFN weights (full resident)
        w1_sb = constp.tile([128, DCH * FF_SH], F32R, tag="w1", name="w1")
        nc.sync.dma_start(w1_sb[:], _rr(w1T))

        ffn_state = {}

        def ffn_load(c):
            t = chkp.tile([128, DCH * NC], F32R, tag="xcat", name=f"cr_{c}")
            nc.sync.dma_start(
                t[:].rearrange("p (u m) -> p u m", u=DCH),
                _rr(cd_c[c][:].rearrange("(u p) m -> p u m", p=128)),
            )
            xc = [t[:, NC * dc:NC * (dc + 1)] for dc in range(DCH)]
            ffn_state[c] = (xc, [])

        def ffn1(c, fb):
            xc, hts = ffn_state[c]
            ps = ps_pp.tile([128, NC], F32, tag="pp", name="ps_f1")
            for dc in range(DCH):
                nc.tensor.matmul(
                    ps[:],
                    w1_sb[:, FF_SH * dc + 128 * fb:FF_SH * dc + 128 * (fb + 1)],
                    xc[dc][:],
                    start=(dc == 0),
                    stop=(dc == DCH - 1),
                )
            ht = chkp.tile([128, NC], F32R, tag=f"h{fb}", name=f"hT{fb}_{c}", bufs=1)
            nc.vector.tensor_relu(ht[:], ps[:])
            hts.append(ht)

        def ffn2(c, ob):
            hts = ffn_state[c][1]
            w2f = workp.tile([128, NFB * 128], F32R, tag="w2f", name="w2f")
            nc.sync.dma_start(
                w2f[:],
                _rr(w2T[:, NFB * 128 * ob:NFB * 128 * (ob + 1)]),
            )
            ps = ps_pp.tile([128, NC], F32, tag="pp", name="ps_f2")
            for fc in range(NFB):
                nc.tensor.matmul(
                    ps[:],
                    w2f[:, 128 * fc:128 * (fc + 1)],
                    hts[fc][:],
                    start=(fc == 0),
                    stop=(fc == NFB - 1),
                )
            o_sb = workp.tile([128, NC], F32, tag="o_sb", name="o_sb")
            nc.vector.tensor_copy(o_sb[:], ps[:])
            nc.gpsimd.dma_start(
                outT[128 * ob:128 * (ob + 1), NC * c:NC * (c + 1)], o_sb[:])

        def ffn_thunks(c):
            ts = [lambda c=c: ffn_load(c)]
            ts += [lambda c=c, fb=fb: ffn1(c, fb) for fb in range(NFB)]
            ts += [lambda c=c, ob=ob: ffn2(c, ob) for ob in range(DCH)]
            return ts

        def pemb_chunks(sc):
            return xcat_load(
                pembT[:, DCH * NC * sc:DCH * NC * (sc + 1)], f"pemb_{sc}")

        # ---- the pipeline ----
        # prologue: projections for self chunk 0
        xc0 = dma_wemb(0)
        proj_q(0, xc0)
        proj_k(0, xc0)
        proj_v(0, xc0)

        # filler schedules per self chunk (n_j = 4, 8, 12, 16)
        def self_fillers(c):
            ts = []
            if c + 1 < NSQ:
                xcn = dma_wemb(c + 1)
                ts += [lambda: proj_q(c + 1, xcn),
                       lambda: proj_k(c + 1, xcn),
                       lambda: proj_v(c + 1, xcn)]
            if c == 1:
                xp0 = pemb_chunks(0)
                ts += [lambda: proj_kc(0, xp0), lambda: proj_vc(0, xp0)]
                xp1 = pemb_chunks(1)
                ts += [lambda: proj_kc(1, xp1), lambda: proj_vc(1, xp1)]
            if c == 2:
                ts += [lambda: qc_proj(0)]
            if c == 3:
                ts += [lambda: qc_proj(1), lambda: qc_proj(2)]
            return ts

        for c in range(NSQ):
            self_c = chkp.tile([128, NC], F32, tag=f"oa{c % 2}", name=f"selfT{c}")
            attention_chunk(self_c[:], qT[:, NC * c:NC * (c + 1)], kT, v65,
                            4 * (c + 1), causal_c=c, fillers=self_fillers(c))
            wd_c.append(allgather(self_c, f"w{c}"))

        # cross chunks with qc/FFN fillers
        def cross_fillers(c):
            ts = []
            if c == 0:
                ts += [lambda: qc_proj(3)]
            if c >= 2:
                ts += ffn_thunks(c - 2)
            return ts

        for c in range(NSQ):
            cross_c = chkp.tile([128, NC], F32, tag=f"oa{c % 2}",
                                name=f"crossT{c}")
            attention_chunk(cross_c[:], qc_t[c][:], kcT, vc65, NSPB,
                            causal_c=None, fillers=cross_fillers(c))
            cd_c[c] = allgather(cross_c, f"c{c}")

        for th in ffn_thunks(NSQ - 2):
            th()
        for th in ffn_thunks(NSQ - 1):
            th()


_CACHED_NC = None


def _build():
    global _CACHED_NC
    if _CACHED_NC is None:
        nc = bacc.Bacc(
            "TRN2",
            target_bir_lowering=False,
            debug=False,
            num_devices=N_CORES,
        )
        with tile.TileContext(nc) as tc:
            decoder_kernel(tc)
        nc.compile()
        _CACHED_NC = nc
    return _CACHED_NC


def _pack_w(wT):
    """[1024, m] -> [128, 8*m]: d-chunk blocks side by side, partition-major."""
    m = wT.shape[1]
    return np.ascontiguousarray(
        wT.reshape(8, 128, m).transpose(1, 0, 2).reshape(128, 8 * m)
        .astype(np.float32))


def _pack_x(xT, nch):
    """[1024, nch*512] -> [128, nch * 8 * 512]: per seq-chunk c, the 8
    feature-blocks of that chunk's columns, contiguous."""
    return np.ascontiguousarray(
        xT.reshape(8, 128, nch, 512).transpose(1, 2, 0, 3)
        .reshape(128, nch * 8 * 512).astype(np.float32))


def make_in_maps(inputs):
    """Host-side prep: transposes + per-core weight slices + prepack."""
    f = np.ascontiguousarray
    wembT = _pack_x(np.asarray(inputs["wemb"], np.float32).T, NSQ)
    pembT = _pack_x(np.asarray(inputs["pemb"], np.float32).T, 2)
    in_maps = []
    for i in range(N_CORES):
        hsl = slice(HD * i, HD * (i + 1))
        fsl = slice(FF_SH * i, FF_SH * (i + 1))
        w2T = np.asarray(inputs["W2"], np.float32)[:, fsl].T  # [512, 1024]
        w2h = f(w2T.reshape(4, 128, 8, 128).transpose(1, 2, 0, 3)
                .reshape(128, 4096))
        in_maps.append({
            "wembT": wembT,
            "pembT": pembT,
            "wqmT": _pack_w(np.asarray(inputs["Wq_m"], np.float32)[hsl, :].T),
            "wkmT": _pack_w(np.asarray(inputs["Wk_m"], np.float32)[hsl, :].T),
            "wvmT": _pack_w(np.asarray(inputs["Wv_m"], np.float32)[hsl, :].T),
            "wqcT": _pack_w(np.asarray(inputs["Wq_c"], np.float32)[hsl, :].T),
            "wkcT": _pack_w(np.asarray(inputs["Wk_c"], np.float32)[hsl, :].T),
            "wvcT": _pack_w(np.asarray(inputs["Wv_c"], np.float32)[hsl, :].T),
            "w1T": _pack_w(np.asarray(inputs["W1"], np.float32)[fsl, :].T),
            "w2T": w2h,
        })
    return in_maps


def kernel(**inputs) -> np.ndarray:
    nc = _build()
    in_maps = make_in_maps(inputs)
    res = run_bass_kernel_spmd(nc, in_maps, core_ids=list(range(N_CORES)))
    acc = np.zeros((D_MODEL, S_W), dtype=np.float64)
    for i in range(N_CORES):
        acc += res.results[i]["outT"]
    return np.ascontiguousarray(acc.T.astype(np.float32))
